# revision 34
# baseline (speedup 1.0000x reference)
"""Trainium2 Bass kernel for nn_ClusterLookup_v2 (vq_codebook).

reference:
    cn = l2norm(clusters, axis=1)        # [K, D]
    xn = l2norm(x, axis=-1)              # [N, D]
    inner = xn @ cn.T                    # [N, K]
    probs = softmax(inner * alpha, 1)    # [N, K]
    loss  = -mean_n(sum_k probs * inner)
    returns (loss, probs)

Data-parallel over 8 NeuronCores (x sharded along N, codebook replicated).
Per core, rows are processed in macro-tiles of 2048 rows = 16 groups of 128
(partition dim), partition-major blocked ("(p a)") so every DMA descriptor
moves >=4KB contiguous.

PE stream is phased to keep the HAM clock-gate warm: PE-mode transposes do
not count as "busy" for the hardware activity monitor, so transposes are
emitted in short bursts (8 per burst, < 3.4us even cold) alternating with
long dense fp32-matmul bursts; the matmul bursts keep K=8/8 and the
transpose bursts are too short to trigger the re-throttle window.

  PE    : per group 4x transpose (identity matmul) + 4x fp32 matmul vs the
          resident normalized-transposed codebook -> raw = x @ cn.T in PSUM
  ACT   : PSUM->SBUF copy of x^T; per-group Exp(raw * alpha/||x||); per-macro
          Exp for the rsqrt seed (single ACT table set -> no reloads)
  DVE   : one-pass ssq via scalar_tensor_tensor(x*1*x, accum_out);
          alpha/||x|| = exp(-0.5*ln~(ssq)+ln(alpha)) where ln~ is the
          exponent-bit approximation, refined by 2 Newton rsqrt iterations;
          softmax denominators (segmented reduce), reciprocal, and the loss
          reduction acc += (alpha/||x||) * sum_k probs*raw
  GPSIMD: probs = e * bcast(1/s) (one op per half-macro)

Loss: host divides the per-core partial by alpha and sums over cores.
Softmax skips max-subtraction (alpha=1, |inner|<=1).
"""

import sys

sys.path.insert(0, "/opt/trn_rl_repo")

import math

import numpy as np

import concourse.bass as bass  # noqa: F401
import concourse.mybir as mybir
import concourse.tile as tile
from concourse import bacc
from concourse.bass_utils import run_bass_kernel_spmd
from concourse.masks import make_identity

N_CORES = 8
D = 512
K = 64
P = 128
DCH = D // P  # 4 contraction chunks

F32 = mybir.dt.float32
I32 = mybir.dt.int32
AF = mybir.ActivationFunctionType
ALU = mybir.AluOpType

NEWTON_ITERS = 2
LN2 = math.log(2.0)


def build_nc(rows: int, g: int = 16, pair: int = 4):
    """Build the per-core Bass module for `rows` rows (multiple of g*128)."""
    n_mac = rows // (g * P)
    assert n_mac * g * P == rows
    assert g % pair == 0

    nc = bacc.Bacc("TRN2", target_bir_lowering=False, debug=False)
    x = nc.dram_tensor("x", [rows, D], F32, kind="ExternalInput")
    clusters = nc.dram_tensor("clusters", [K, D], F32, kind="ExternalInput")
    alphav = nc.dram_tensor("alphav", [P, 1], F32, kind="ExternalInput")
    probs = nc.dram_tensor("probs", [rows, K], F32, kind="ExternalOutput")
    lsum = nc.dram_tensor("lsum", [1, 1], F32, kind="ExternalOutput")

    with tile.TileContext(nc) as tc:
        with (
            tc.tile_pool(name="singles", bufs=1) as singles,
            tc.tile_pool(name="xin", bufs=2) as xin_pool,
            tc.tile_pool(name="xt", bufs=12) as xt_pool,
            tc.tile_pool(name="ev", bufs=2) as ev_pool,
            tc.tile_pool(name="stage", bufs=2) as stage_pool,
            tc.tile_pool(name="ps_xt", bufs=4, space="PSUM") as ps_xt_pool,
            tc.tile_pool(name="ps_raw", bufs=4, space="PSUM") as ps_raw_pool,
        ):
            # ---------------- setup (once per kernel) ----------------
            ident = singles.tile([P, P], F32)
            make_identity(nc, ident[:])
            alpha_sb = singles.tile([P, 1], F32)
            nc.sync.dma_start(out=alpha_sb[:], in_=alphav[:, :])
            ones = singles.tile([P, 1], F32)
            nc.vector.memset(ones[:], 1.0)

            # ln-approx constants:  arg = -0.5*ln(ssq) + ln(alpha)
            #   ln(v) ~ LN2 * (intbits(v)/2^23 - 126.94269504)
            # so arg = c1*intbits(ssq) + lnA2, with
            #   c1 = -0.5*LN2/2^23,  lnA2 = 0.5*LN2*126.94269504 + ln(alpha)
            ln_alpha = singles.tile([P, 1], F32)
            nc.scalar.activation(ln_alpha[:], alpha_sb[:], AF.Ln)
            lnA2 = singles.tile([P, 1], F32)
            nc.vector.tensor_scalar(
                out=lnA2[:], in0=ln_alpha[:], scalar1=1.0,
                scalar2=float(0.5 * LN2 * 126.94269504),
                op0=ALU.mult, op1=ALU.add,
            )
            # Newton constant: -0.5/alpha^2 (per-partition AP)
            a_m05i2 = singles.tile([P, 1], F32)
            nc.vector.reciprocal(a_m05i2[:], alpha_sb[:])
            nc.vector.tensor_mul(a_m05i2[:], a_m05i2[:], a_m05i2[:])
            nc.vector.tensor_scalar_mul(a_m05i2[:], a_m05i2[:], -0.5)

            # codebook: load natural [64, 512], l2-normalize rows, transpose
            c_nat = singles.tile([K, D], F32)
            nc.sync.dma_start(out=c_nat[:], in_=clusters[:, :])
            c_sq = singles.tile([K, D], F32)
            nc.vector.tensor_mul(c_sq[:], c_nat[:], c_nat[:])
            c_ssq = singles.tile([K, 1], F32)
            nc.vector.tensor_reduce(
                c_ssq[:], c_sq[:], axis=mybir.AxisListType.X, op=ALU.add
            )
            # 1/||c|| = exp(-0.5*ln(ssq)) (Ln/Exp: same ACT set as main loop)
            nc.scalar.activation(c_ssq[:], c_ssq[:], AF.Ln)
            nc.scalar.activation(c_ssq[:], c_ssq[:], AF.Exp, scale=-0.5)
            nc.vector.tensor_scalar_mul(c_nat[:], c_nat[:], c_ssq[:])
            cT = singles.tile([P, DCH, K], F32)
            for j in range(DCH):
                ps_ct = ps_xt_pool.tile([P, K], F32, tag="ps_xt")
                nc.tensor.transpose(
                    ps_ct[:], c_nat[:, j * P:(j + 1) * P], ident[:K, :K]
                )
                nc.scalar.copy(out=cT[:, j, :], in_=ps_ct[:])

            acc = singles.tile([P, g], F32)
            nc.vector.memset(acc[:], 0.0)

            # ---------------- main loop ----------------
            # Flat software pipeline over (macro, pair) units.  For each unit
            # we emit stats + transposes + PSUM->SBUF copies, then the raw
            # matmuls of the *previous* unit, so the PE stream alternates
            # short transpose bursts with dense matmul bursts (HAM-friendly)
            # with no matmul-sparse stretch at macro boundaries.  A macro's
            # tail (softmax etc.) is emitted right after its last matmuls.
            state: dict = {}

            def start_macro(m):
                r0 = m * g * P
                st = {}
                st["x_t"] = xin_pool.tile([P, g, D], F32, tag="x", name="x_t")
                src_ap = x[r0:r0 + g * P, :].rearrange("(p a) d -> p a d", p=P)
                step = pair if m == 0 else g // 2
                for a0 in range(0, g, step):
                    nc.sync.dma_start(
                        out=st["x_t"][:, a0:a0 + step, :],
                        in_=src_ap[:, a0:a0 + step, :],
                    )
                h = g // 2
                st["ps_raw"] = [
                    ps_raw_pool.tile([P, h * K], F32, tag="raw", name="ps_raw")
                    for _ in range(2)
                ]
                for t_ in ("ssq", "arg", "z", "t1", "rna", "s", "r", "lr"):
                    st[t_] = stage_pool.tile([P, g], F32, tag=t_, name=t_)
                st["gar"] = stage_pool.tile([P, D], F32, tag="gar", name="gar")
                st["e_t"] = ev_pool.tile([P, g, K], F32, tag="e", name="e")
                st["out_t"] = ev_pool.tile([P, g, K], F32, tag="po", name="po")
                st["pi2"] = ev_pool.tile([P, g, K], F32, tag="pi", name="pi")
                state[m] = st
                return st

            def emit_unit_front(m, q):
                st = state[m]
                x_t, ps_raw = st["x_t"], st["ps_raw"]
                xts = []
                for gi in range(q * pair, (q + 1) * pair):
                    xg = x_t[:, gi, :]
                    # one-pass row ssq (DVE)
                    nc.vector.scalar_tensor_tensor(
                        out=st["gar"][:], in0=xg, scalar=1.0, in1=xg,
                        op0=ALU.mult, op1=ALU.mult,
                        accum_out=st["ssq"][:, gi:gi + 1],
                    )
                    # transpose burst (PE)
                    ps_xt = ps_xt_pool.tile([P, D], F32, tag="ps_xt")
                    for j in range(DCH):
                        nc.tensor.transpose(
                            ps_xt[:, j * P:(j + 1) * P],
                            xg[:, j * P:(j + 1) * P],
                            ident[:],
                        )
                    xt_sb = xt_pool.tile([P, D], F32, tag="xt")
                    nc.scalar.copy(out=xt_sb[:], in_=ps_xt[:])
                    xts.append((gi, xt_sb))

                h = g // 2

                def emit_mms():
                    for gi, xt_sb in xts:
                        raw = ps_raw[gi // h][:, (gi % h) * K:(gi % h + 1) * K]
                        for j in range(DCH):
                            nc.tensor.matmul(
                                raw,
                                lhsT=xt_sb[:, j * P:(j + 1) * P],
                                rhs=cT[:, j, :],
                                start=(j == 0), stop=(j == DCH - 1),
                            )
                return emit_mms

            def emit_rn_chain(m, sl=slice(None)):
                # rsqrt: arg = c1*intbits(ssq) + lnA2 ; z0 = exp(arg);
                # Newton z <- z*(1.5 + (-0.5/a^2)*ssq*z^2)  (z ~ alpha/||x||)
                st = state[m]
                nc.vector.tensor_scalar(
                    out=st["arg"][:, sl], in0=st["ssq"][:, sl].bitcast(I32),
                    scalar1=float(-0.5 * LN2 / (1 << 23)), scalar2=lnA2[:],
                    op0=ALU.mult, op1=ALU.add,
                )
                nc.scalar.activation(st["z"][:, sl], st["arg"][:, sl], AF.Exp)
                for it in range(NEWTON_ITERS):
                    dst = st["rna"] if it == NEWTON_ITERS - 1 else st["z"]
                    nc.vector.tensor_mul(
                        st["t1"][:, sl], st["z"][:, sl], st["z"][:, sl]
                    )
                    nc.vector.tensor_mul(
                        st["t1"][:, sl], st["t1"][:, sl], st["ssq"][:, sl]
                    )
                    nc.vector.tensor_scalar(
                        out=st["t1"][:, sl], in0=st["t1"][:, sl],
                        scalar1=a_m05i2[:],
                        scalar2=1.5, op0=ALU.mult, op1=ALU.add,
                    )
                    nc.vector.tensor_mul(
                        dst[:, sl], st["z"][:, sl], st["t1"][:, sl]
                    )

            def emit_tail_half(m, hi):
                st = state[m] if hi == 0 else state.pop(m)
                r0 = m * g * P
                h = g // 2
                sl = slice(hi * h, (hi + 1) * h)
                raw = st["ps_raw"][hi]
                e_t, out_t, pi2 = st["e_t"], st["out_t"], st["pi2"]
                for li in range(h):
                    gi = hi * h + li
                    # e = exp(raw * alpha/||x||)  (ACT, PSUM src)
                    nc.scalar.activation(
                        out=e_t[:, gi, :],
                        in_=raw[:, li * K:(li + 1) * K],
                        func=AF.Exp,
                        scale=st["rna"][:, gi:gi + 1],
                    )
                # softmax denominators + reciprocal (DVE)
                nc.vector.tensor_reduce(
                    st["s"][:, sl], e_t[:, sl, :],
                    axis=mybir.AxisListType.X, op=ALU.add,
                )
                nc.vector.reciprocal(st["r"][:, sl], st["s"][:, sl])
                # probs = e * bcast(1/s)   (GPSIMD)
                r_bcast = st["r"][:, sl].unsqueeze(2).to_broadcast([P, h, K])
                nc.gpsimd.tensor_tensor(
                    out_t[:, sl, :], e_t[:, sl, :], r_bcast, op=ALU.mult
                )
                # loss: acc += rna * sum_k probs*raw   (DVE)
                nc.vector.tensor_mul(
                    pi2[:, sl, :].rearrange("p a k -> p (a k)"),
                    out_t[:, sl, :].rearrange("p a k -> p (a k)"),
                    raw[:],
                )
                nc.vector.tensor_reduce(
                    st["lr"][:, sl], pi2[:, sl, :],
                    axis=mybir.AxisListType.X, op=ALU.add,
                )
                nc.vector.tensor_mul(
                    st["lr"][:, sl], st["lr"][:, sl], st["rna"][:, sl]
                )
                nc.vector.tensor_add(acc[:, sl], acc[:, sl], st["lr"][:, sl])
                # probs out (ACT HWDGE ring): 2KB contiguous per partition
                pr = probs[r0:r0 + g * P, :].rearrange(
                    "(p b a) k -> p b a k", p=P, b=2
                )
                nc.scalar.dma_start(out=pr[:, hi], in_=out_t[:, sl, :])

            n_pairs = g // pair
            h = g // 2
            hp = n_pairs // 2  # pairs per half
            units = [(m, q) for m in range(n_mac) for q in range(n_pairs)]
            from collections import deque
            pend = deque()  # (m, q, emit_mms), depth-2 deferral
            half1_of = None  # macro whose tail-half-1 is deferred
            last = n_mac - 1

            def drain_one():
                nonlocal half1_of
                pm, pq, pfn = pend.popleft()
                pfn()
                if half1_of is not None:
                    emit_tail_half(half1_of, 1)
                    half1_of = None
                if pq == n_pairs - 1:
                    if pm != last:
                        emit_tail_half(pm, 0)
                    half1_of = pm
                elif pm == last and pq == hp - 1:
                    # last macro: first half fully matmul'd, rna(half0)
                    # ready -> drain it early
                    emit_tail_half(pm, 0)

            for m, q in units:
                if q == 0 and m not in state:
                    start_macro(m)
                emit_mms = emit_unit_front(m, q)
                if m == last and q == hp - 1:
                    emit_rn_chain(m, slice(0, h))
                elif m == last and q == n_pairs - 1:
                    emit_rn_chain(m, slice(h, g))
                elif m != last and q == n_pairs - 1:
                    emit_rn_chain(m)
                pend.append((m, q, emit_mms))
                if len(pend) > 3:
                    drain_one()
            while pend:
                drain_one()
            if half1_of is not None:
                emit_tail_half(half1_of, 1)

            # ---------------- loss partial ----------------
            accv = singles.tile([P, 1], F32)
            nc.vector.tensor_reduce(
                accv[:], acc[:], axis=mybir.AxisListType.X, op=ALU.add
            )
            ps_fin = ps_raw_pool.tile([1, 1], F32, tag="raw", name="ps_fin")
            nc.tensor.matmul(
                ps_fin[:], lhsT=accv[:], rhs=ones[:], start=True, stop=True
            )
            ls_sb = singles.tile([1, 1], F32)
            nc.scalar.copy(out=ls_sb[:], in_=ps_fin[:])
            nc.sync.dma_start(out=lsum[:, :], in_=ls_sb[:])

    nc.compile()
    return nc


_NC_CACHE: dict = {}


def _get_nc(rows: int):
    if rows not in _NC_CACHE:
        _NC_CACHE[rows] = build_nc(rows)
    return _NC_CACHE[rows]


def kernel(x, clusters, alpha):
    x = np.ascontiguousarray(np.asarray(x, dtype=np.float32))
    clusters = np.ascontiguousarray(np.asarray(clusters, dtype=np.float32))
    alpha_f = np.float32(np.asarray(alpha).reshape(()))

    n = x.shape[0]
    rows = n // N_CORES
    nc = _get_nc(rows)

    alphav = np.full((P, 1), alpha_f, dtype=np.float32)
    in_maps = [
        {"x": x[i * rows:(i + 1) * rows], "clusters": clusters, "alphav": alphav}
        for i in range(N_CORES)
    ]
    res = run_bass_kernel_spmd(nc, in_maps, list(range(N_CORES)))
    probs = np.concatenate([res.results[i]["probs"] for i in range(N_CORES)], axis=0)
    total = sum(float(res.results[i]["lsum"][0, 0]) for i in range(N_CORES))
    loss = np.float32(-total / (n * float(alpha_f)))
    return loss, probs


# revision 35
# speedup vs baseline: 1.0046x; 1.0046x over previous
"""Trainium2 Bass kernel for nn_ClusterLookup_v2 (vq_codebook).

reference:
    cn = l2norm(clusters, axis=1)        # [K, D]
    xn = l2norm(x, axis=-1)              # [N, D]
    inner = xn @ cn.T                    # [N, K]
    probs = softmax(inner * alpha, 1)    # [N, K]
    loss  = -mean_n(sum_k probs * inner)
    returns (loss, probs)

Data-parallel over 8 NeuronCores (x sharded along N, codebook replicated).
Per core, rows are processed in macro-tiles of 2048 rows = 16 groups of 128
(partition dim), partition-major blocked ("(p a)") so every DMA descriptor
moves >=4KB contiguous.

PE stream is phased to keep the HAM clock-gate warm: PE-mode transposes do
not count as "busy" for the hardware activity monitor, so transposes are
emitted in short bursts (8 per burst, < 3.4us even cold) alternating with
long dense fp32-matmul bursts; the matmul bursts keep K=8/8 and the
transpose bursts are too short to trigger the re-throttle window.

  PE    : per group 4x transpose (identity matmul) + 4x fp32 matmul vs the
          resident normalized-transposed codebook -> raw = x @ cn.T in PSUM
  ACT   : PSUM->SBUF copy of x^T; per-group Exp(raw * alpha/||x||); per-macro
          Exp for the rsqrt seed (single ACT table set -> no reloads)
  DVE   : one-pass ssq via scalar_tensor_tensor(x*1*x, accum_out);
          alpha/||x|| = exp(-0.5*ln~(ssq)+ln(alpha)) where ln~ is the
          exponent-bit approximation, refined by 2 Newton rsqrt iterations;
          softmax denominators (segmented reduce), reciprocal, and the loss
          reduction acc += (alpha/||x||) * sum_k probs*raw
  GPSIMD: probs = e * bcast(1/s) (one op per half-macro)

Loss: host divides the per-core partial by alpha and sums over cores.
Softmax skips max-subtraction (alpha=1, |inner|<=1).
"""

import sys

sys.path.insert(0, "/opt/trn_rl_repo")

import math

import numpy as np

import concourse.bass as bass  # noqa: F401
import concourse.mybir as mybir
import concourse.tile as tile
from concourse import bacc
from concourse.bass_utils import run_bass_kernel_spmd
from concourse.masks import make_identity

N_CORES = 8
D = 512
K = 64
P = 128
DCH = D // P  # 4 contraction chunks

F32 = mybir.dt.float32
I32 = mybir.dt.int32
AF = mybir.ActivationFunctionType
ALU = mybir.AluOpType

NEWTON_ITERS = 2
LN2 = math.log(2.0)


def build_nc(rows: int, g: int = 16, pair: int = 4):
    """Build the per-core Bass module for `rows` rows (multiple of g*128)."""
    n_mac = rows // (g * P)
    assert n_mac * g * P == rows
    assert g % pair == 0

    nc = bacc.Bacc("TRN2", target_bir_lowering=False, debug=False)
    x = nc.dram_tensor("x", [rows, D], F32, kind="ExternalInput")
    clusters = nc.dram_tensor("clusters", [K, D], F32, kind="ExternalInput")
    alphav = nc.dram_tensor("alphav", [P, 1], F32, kind="ExternalInput")
    probs = nc.dram_tensor("probs", [rows, K], F32, kind="ExternalOutput")
    lsum = nc.dram_tensor("lsum", [1, 1], F32, kind="ExternalOutput")

    with tile.TileContext(nc) as tc:
        with (
            tc.tile_pool(name="singles", bufs=1) as singles,
            tc.tile_pool(name="xin", bufs=2) as xin_pool,
            tc.tile_pool(name="xt", bufs=12) as xt_pool,
            tc.tile_pool(name="ev", bufs=2) as ev_pool,
            tc.tile_pool(name="stage", bufs=2) as stage_pool,
            tc.tile_pool(name="ps_xt", bufs=4, space="PSUM") as ps_xt_pool,
            tc.tile_pool(name="ps_raw", bufs=4, space="PSUM") as ps_raw_pool,
        ):
            # ---------------- setup (once per kernel) ----------------
            ident = singles.tile([P, P], F32)
            make_identity(nc, ident[:])
            alpha_sb = singles.tile([P, 1], F32)
            nc.sync.dma_start(out=alpha_sb[:], in_=alphav[:, :])
            ones = singles.tile([P, 1], F32)
            nc.vector.memset(ones[:], 1.0)

            # ln-approx constants:  arg = -0.5*ln(ssq) + ln(alpha)
            #   ln(v) ~ LN2 * (intbits(v)/2^23 - 126.94269504)
            # so arg = c1*intbits(ssq) + lnA2, with
            #   c1 = -0.5*LN2/2^23,  lnA2 = 0.5*LN2*126.94269504 + ln(alpha)
            ln_alpha = singles.tile([P, 1], F32)
            nc.scalar.activation(ln_alpha[:], alpha_sb[:], AF.Ln)
            lnA2 = singles.tile([P, 1], F32)
            nc.vector.tensor_scalar(
                out=lnA2[:], in0=ln_alpha[:], scalar1=1.0,
                scalar2=float(0.5 * LN2 * 126.94269504),
                op0=ALU.mult, op1=ALU.add,
            )
            # Newton constant: -0.5/alpha^2 (per-partition AP)
            a_m05i2 = singles.tile([P, 1], F32)
            nc.vector.reciprocal(a_m05i2[:], alpha_sb[:])
            nc.vector.tensor_mul(a_m05i2[:], a_m05i2[:], a_m05i2[:])
            nc.vector.tensor_scalar_mul(a_m05i2[:], a_m05i2[:], -0.5)

            # codebook: load natural [64, 512], l2-normalize rows, transpose
            c_nat = singles.tile([K, D], F32)
            nc.sync.dma_start(out=c_nat[:], in_=clusters[:, :])
            c_sq = singles.tile([K, D], F32)
            nc.vector.tensor_mul(c_sq[:], c_nat[:], c_nat[:])
            c_ssq = singles.tile([K, 1], F32)
            nc.vector.tensor_reduce(
                c_ssq[:], c_sq[:], axis=mybir.AxisListType.X, op=ALU.add
            )
            # 1/||c|| = exp(-0.5*ln(ssq)) (Ln/Exp: same ACT set as main loop)
            nc.scalar.activation(c_ssq[:], c_ssq[:], AF.Ln)
            nc.scalar.activation(c_ssq[:], c_ssq[:], AF.Exp, scale=-0.5)
            nc.vector.tensor_scalar_mul(c_nat[:], c_nat[:], c_ssq[:])
            cT = singles.tile([P, DCH, K], F32)
            for j in range(DCH):
                ps_ct = ps_xt_pool.tile([P, K], F32, tag="ps_xt")
                nc.tensor.transpose(
                    ps_ct[:], c_nat[:, j * P:(j + 1) * P], ident[:K, :K]
                )
                nc.scalar.copy(out=cT[:, j, :], in_=ps_ct[:])

            acc = singles.tile([P, g], F32)
            nc.vector.memset(acc[:], 0.0)

            # ---------------- main loop ----------------
            # Flat software pipeline over (macro, pair) units.  For each unit
            # we emit stats + transposes + PSUM->SBUF copies, then the raw
            # matmuls of the *previous* unit, so the PE stream alternates
            # short transpose bursts with dense matmul bursts (HAM-friendly)
            # with no matmul-sparse stretch at macro boundaries.  A macro's
            # tail (softmax etc.) is emitted right after its last matmuls.
            state: dict = {}

            def start_macro(m):
                r0 = m * g * P
                st = {}
                st["x_t"] = xin_pool.tile([P, g, D], F32, tag="x", name="x_t")
                src_ap = x[r0:r0 + g * P, :].rearrange("(p a) d -> p a d", p=P)
                step = pair if m == 0 else g // 2
                for a0 in range(0, g, step):
                    nc.sync.dma_start(
                        out=st["x_t"][:, a0:a0 + step, :],
                        in_=src_ap[:, a0:a0 + step, :],
                    )
                h = g // 2
                st["ps_raw"] = [
                    ps_raw_pool.tile([P, h * K], F32, tag="raw", name="ps_raw")
                    for _ in range(2)
                ]
                for t_ in ("ssq", "arg", "z", "t1", "rna", "s", "r", "lr"):
                    st[t_] = stage_pool.tile([P, g], F32, tag=t_, name=t_)
                st["gar"] = stage_pool.tile([P, D], F32, tag="gar", name="gar")
                st["e_t"] = ev_pool.tile([P, g, K], F32, tag="e", name="e")
                st["out_t"] = ev_pool.tile([P, g, K], F32, tag="po", name="po")
                st["pi2"] = ev_pool.tile([P, g, K], F32, tag="pi", name="pi")
                state[m] = st
                return st

            def emit_unit_front(m, q):
                st = state[m]
                x_t, ps_raw = st["x_t"], st["ps_raw"]
                xts = []
                for gi in range(q * pair, (q + 1) * pair):
                    xg = x_t[:, gi, :]
                    # one-pass row ssq (DVE)
                    nc.vector.scalar_tensor_tensor(
                        out=st["gar"][:], in0=xg, scalar=1.0, in1=xg,
                        op0=ALU.mult, op1=ALU.mult,
                        accum_out=st["ssq"][:, gi:gi + 1],
                    )
                    # transpose burst (PE)
                    ps_xt = ps_xt_pool.tile([P, D], F32, tag="ps_xt")
                    for j in range(DCH):
                        nc.tensor.transpose(
                            ps_xt[:, j * P:(j + 1) * P],
                            xg[:, j * P:(j + 1) * P],
                            ident[:],
                        )
                    xt_sb = xt_pool.tile([P, D], F32, tag="xt")
                    nc.scalar.copy(out=xt_sb[:], in_=ps_xt[:])
                    xts.append((gi, xt_sb))

                h = g // 2

                def emit_mms():
                    for gi, xt_sb in xts:
                        raw = ps_raw[gi // h][:, (gi % h) * K:(gi % h + 1) * K]
                        for j in range(DCH):
                            nc.tensor.matmul(
                                raw,
                                lhsT=xt_sb[:, j * P:(j + 1) * P],
                                rhs=cT[:, j, :],
                                start=(j == 0), stop=(j == DCH - 1),
                            )
                return emit_mms

            def emit_rn_chain(m, sl=slice(None)):
                # rsqrt: arg = c1*intbits(ssq) + lnA2 ; z0 = exp(arg);
                # Newton z <- z*(1.5 + (-0.5/a^2)*ssq*z^2)  (z ~ alpha/||x||)
                st = state[m]
                nc.vector.tensor_scalar(
                    out=st["arg"][:, sl], in0=st["ssq"][:, sl].bitcast(I32),
                    scalar1=float(-0.5 * LN2 / (1 << 23)), scalar2=lnA2[:],
                    op0=ALU.mult, op1=ALU.add,
                )
                nc.scalar.activation(st["z"][:, sl], st["arg"][:, sl], AF.Exp)
                for it in range(NEWTON_ITERS):
                    dst = st["rna"] if it == NEWTON_ITERS - 1 else st["z"]
                    nc.vector.tensor_mul(
                        st["t1"][:, sl], st["z"][:, sl], st["z"][:, sl]
                    )
                    nc.vector.tensor_mul(
                        st["t1"][:, sl], st["t1"][:, sl], st["ssq"][:, sl]
                    )
                    nc.vector.tensor_scalar(
                        out=st["t1"][:, sl], in0=st["t1"][:, sl],
                        scalar1=a_m05i2[:],
                        scalar2=1.5, op0=ALU.mult, op1=ALU.add,
                    )
                    nc.vector.tensor_mul(
                        dst[:, sl], st["z"][:, sl], st["t1"][:, sl]
                    )

            def emit_tail_half(m, hi):
                st = state[m] if hi == 0 else state.pop(m)
                r0 = m * g * P
                h = g // 2
                sl = slice(hi * h, (hi + 1) * h)
                raw = st["ps_raw"][hi]
                e_t, out_t, pi2 = st["e_t"], st["out_t"], st["pi2"]
                for li in range(h):
                    gi = hi * h + li
                    # e = exp(raw * alpha/||x||)  (ACT, PSUM src)
                    nc.scalar.activation(
                        out=e_t[:, gi, :],
                        in_=raw[:, li * K:(li + 1) * K],
                        func=AF.Exp,
                        scale=st["rna"][:, gi:gi + 1],
                    )
                # softmax denominators + reciprocal (DVE)
                nc.vector.tensor_reduce(
                    st["s"][:, sl], e_t[:, sl, :],
                    axis=mybir.AxisListType.X, op=ALU.add,
                )
                nc.vector.reciprocal(st["r"][:, sl], st["s"][:, sl])
                # probs = e * bcast(1/s)   (GPSIMD)
                r_bcast = st["r"][:, sl].unsqueeze(2).to_broadcast([P, h, K])
                nc.gpsimd.tensor_tensor(
                    out_t[:, sl, :], e_t[:, sl, :], r_bcast, op=ALU.mult
                )
                # loss: acc += rna * sum_k probs*raw   (DVE)
                nc.vector.tensor_mul(
                    pi2[:, sl, :].rearrange("p a k -> p (a k)"),
                    out_t[:, sl, :].rearrange("p a k -> p (a k)"),
                    raw[:],
                )
                nc.vector.tensor_reduce(
                    st["lr"][:, sl], pi2[:, sl, :],
                    axis=mybir.AxisListType.X, op=ALU.add,
                )
                nc.vector.tensor_mul(
                    st["lr"][:, sl], st["lr"][:, sl], st["rna"][:, sl]
                )
                nc.vector.tensor_add(acc[:, sl], acc[:, sl], st["lr"][:, sl])
                # probs out (ACT HWDGE ring): 2KB contiguous per partition
                pr = probs[r0:r0 + g * P, :].rearrange(
                    "(p b a) k -> p b a k", p=P, b=2
                )
                nc.scalar.dma_start(out=pr[:, hi], in_=out_t[:, sl, :])

            n_pairs = g // pair
            h = g // 2
            hp = n_pairs // 2  # pairs per half
            units = [(m, q) for m in range(n_mac) for q in range(n_pairs)]
            from collections import deque
            pend = deque()  # (m, q, emit_mms), depth-2 deferral
            half1_of = None  # macro whose tail-half-1 is deferred
            last = n_mac - 1

            def drain_one():
                nonlocal half1_of
                pm, pq, pfn = pend.popleft()
                pfn()
                if half1_of is not None:
                    emit_tail_half(half1_of, 1)
                    half1_of = None
                if pq == n_pairs - 1:
                    if pm != last:
                        emit_tail_half(pm, 0)
                    half1_of = pm
                elif pm == last and pq == hp - 1:
                    # last macro: first half fully matmul'd, rna(half0)
                    # ready -> drain it early
                    emit_tail_half(pm, 0)

            for m, q in units:
                if q == 0 and m not in state:
                    start_macro(m)
                emit_mms = emit_unit_front(m, q)
                if m == last and q == hp - 1:
                    emit_rn_chain(m, slice(0, h))
                elif m == last and q == n_pairs - 1:
                    emit_rn_chain(m, slice(h, g))
                elif m != last and q == n_pairs - 1:
                    emit_rn_chain(m)
                pend.append((m, q, emit_mms))
                if len(pend) > 2:
                    drain_one()
            while pend:
                drain_one()
            if half1_of is not None:
                emit_tail_half(half1_of, 1)

            # ---------------- loss partial ----------------
            accv = singles.tile([P, 1], F32)
            nc.vector.tensor_reduce(
                accv[:], acc[:], axis=mybir.AxisListType.X, op=ALU.add
            )
            ps_fin = ps_raw_pool.tile([1, 1], F32, tag="raw", name="ps_fin")
            nc.tensor.matmul(
                ps_fin[:], lhsT=accv[:], rhs=ones[:], start=True, stop=True
            )
            ls_sb = singles.tile([1, 1], F32)
            nc.scalar.copy(out=ls_sb[:], in_=ps_fin[:])
            nc.sync.dma_start(out=lsum[:, :], in_=ls_sb[:])

    nc.compile()
    return nc


_NC_CACHE: dict = {}


def _get_nc(rows: int):
    if rows not in _NC_CACHE:
        _NC_CACHE[rows] = build_nc(rows)
    return _NC_CACHE[rows]


def kernel(x, clusters, alpha):
    x = np.ascontiguousarray(np.asarray(x, dtype=np.float32))
    clusters = np.ascontiguousarray(np.asarray(clusters, dtype=np.float32))
    alpha_f = np.float32(np.asarray(alpha).reshape(()))

    n = x.shape[0]
    rows = n // N_CORES
    nc = _get_nc(rows)

    alphav = np.full((P, 1), alpha_f, dtype=np.float32)
    in_maps = [
        {"x": x[i * rows:(i + 1) * rows], "clusters": clusters, "alphav": alphav}
        for i in range(N_CORES)
    ]
    res = run_bass_kernel_spmd(nc, in_maps, list(range(N_CORES)))
    probs = np.concatenate([res.results[i]["probs"] for i in range(N_CORES)], axis=0)
    total = sum(float(res.results[i]["lsum"][0, 0]) for i in range(N_CORES))
    loss = np.float32(-total / (n * float(alpha_f)))
    return loss, probs


# revision 36
# speedup vs baseline: 1.0169x; 1.0122x over previous
"""Trainium2 Bass kernel for nn_ClusterLookup_v2 (vq_codebook).

reference:
    cn = l2norm(clusters, axis=1)        # [K, D]
    xn = l2norm(x, axis=-1)              # [N, D]
    inner = xn @ cn.T                    # [N, K]
    probs = softmax(inner * alpha, 1)    # [N, K]
    loss  = -mean_n(sum_k probs * inner)
    returns (loss, probs)

Data-parallel over 8 NeuronCores (x sharded along N, codebook replicated).
Per core, rows are processed in macro-tiles of 2048 rows = 16 groups of 128
(partition dim), partition-major blocked ("(p a)") so every DMA descriptor
moves >=4KB contiguous.

PE stream is phased to keep the HAM clock-gate warm: PE-mode transposes do
not count as "busy" for the hardware activity monitor, so transposes are
emitted in short bursts (8 per burst, < 3.4us even cold) alternating with
long dense fp32-matmul bursts; the matmul bursts keep K=8/8 and the
transpose bursts are too short to trigger the re-throttle window.

  PE    : per group 4x transpose (identity matmul) + 4x fp32 matmul vs the
          resident normalized-transposed codebook -> raw = x @ cn.T in PSUM
  ACT   : PSUM->SBUF copy of x^T; per-group Exp(raw * alpha/||x||); per-macro
          Exp for the rsqrt seed (single ACT table set -> no reloads)
  DVE   : one-pass ssq via scalar_tensor_tensor(x*1*x, accum_out);
          alpha/||x|| = exp(-0.5*ln~(ssq)+ln(alpha)) where ln~ is the
          exponent-bit approximation, refined by 2 Newton rsqrt iterations;
          softmax denominators (segmented reduce), reciprocal, and the loss
          reduction acc += (alpha/||x||) * sum_k probs*raw
  GPSIMD: probs = e * bcast(1/s) (one op per half-macro)

Loss: host divides the per-core partial by alpha and sums over cores.
Softmax skips max-subtraction (alpha=1, |inner|<=1).
"""

import sys

sys.path.insert(0, "/opt/trn_rl_repo")

import math

import numpy as np

import concourse.bass as bass  # noqa: F401
import concourse.mybir as mybir
import concourse.tile as tile
from concourse import bacc
from concourse.bass_utils import run_bass_kernel_spmd
from concourse.masks import make_identity

N_CORES = 8
D = 512
K = 64
P = 128
DCH = D // P  # 4 contraction chunks

F32 = mybir.dt.float32
I32 = mybir.dt.int32
AF = mybir.ActivationFunctionType
ALU = mybir.AluOpType

NEWTON_ITERS = 2
LN2 = math.log(2.0)


def build_nc(rows: int, g: int = 16, pair: int = 4):
    """Build the per-core Bass module for `rows` rows (multiple of g*128)."""
    n_mac = rows // (g * P)
    assert n_mac * g * P == rows
    assert g % pair == 0

    nc = bacc.Bacc("TRN2", target_bir_lowering=False, debug=False)
    x = nc.dram_tensor("x", [rows, D], F32, kind="ExternalInput")
    clusters = nc.dram_tensor("clusters", [K, D], F32, kind="ExternalInput")
    alphav = nc.dram_tensor("alphav", [P, 1], F32, kind="ExternalInput")
    probs = nc.dram_tensor("probs", [rows, K], F32, kind="ExternalOutput")
    lsum = nc.dram_tensor("lsum", [1, 1], F32, kind="ExternalOutput")

    with tile.TileContext(nc) as tc:
        with (
            tc.tile_pool(name="singles", bufs=1) as singles,
            tc.tile_pool(name="xin", bufs=2) as xin_pool,
            tc.tile_pool(name="xt", bufs=12) as xt_pool,
            tc.tile_pool(name="ev", bufs=3) as ev_pool,
            tc.tile_pool(name="stage", bufs=3) as stage_pool,
            tc.tile_pool(name="ps_xt", bufs=4, space="PSUM") as ps_xt_pool,
            tc.tile_pool(name="ps_raw", bufs=4, space="PSUM") as ps_raw_pool,
        ):
            # ---------------- setup (once per kernel) ----------------
            ident = singles.tile([P, P], F32)
            make_identity(nc, ident[:])
            alpha_sb = singles.tile([P, 1], F32)
            nc.sync.dma_start(out=alpha_sb[:], in_=alphav[:, :])
            ones = singles.tile([P, 1], F32)
            nc.vector.memset(ones[:], 1.0)

            # ln-approx constants:  arg = -0.5*ln(ssq) + ln(alpha)
            #   ln(v) ~ LN2 * (intbits(v)/2^23 - 126.94269504)
            # so arg = c1*intbits(ssq) + lnA2, with
            #   c1 = -0.5*LN2/2^23,  lnA2 = 0.5*LN2*126.94269504 + ln(alpha)
            ln_alpha = singles.tile([P, 1], F32)
            nc.scalar.activation(ln_alpha[:], alpha_sb[:], AF.Ln)
            lnA2 = singles.tile([P, 1], F32)
            nc.vector.tensor_scalar(
                out=lnA2[:], in0=ln_alpha[:], scalar1=1.0,
                scalar2=float(0.5 * LN2 * 126.94269504),
                op0=ALU.mult, op1=ALU.add,
            )
            # Newton constant: -0.5/alpha^2 (per-partition AP)
            a_m05i2 = singles.tile([P, 1], F32)
            nc.vector.reciprocal(a_m05i2[:], alpha_sb[:])
            nc.vector.tensor_mul(a_m05i2[:], a_m05i2[:], a_m05i2[:])
            nc.vector.tensor_scalar_mul(a_m05i2[:], a_m05i2[:], -0.5)

            # codebook: load natural [64, 512], l2-normalize rows, transpose
            c_nat = singles.tile([K, D], F32)
            nc.sync.dma_start(out=c_nat[:], in_=clusters[:, :])
            c_sq = singles.tile([K, D], F32)
            nc.vector.tensor_mul(c_sq[:], c_nat[:], c_nat[:])
            c_ssq = singles.tile([K, 1], F32)
            nc.vector.tensor_reduce(
                c_ssq[:], c_sq[:], axis=mybir.AxisListType.X, op=ALU.add
            )
            # 1/||c|| = exp(-0.5*ln(ssq)) (Ln/Exp: same ACT set as main loop)
            nc.scalar.activation(c_ssq[:], c_ssq[:], AF.Ln)
            nc.scalar.activation(c_ssq[:], c_ssq[:], AF.Exp, scale=-0.5)
            nc.vector.tensor_scalar_mul(c_nat[:], c_nat[:], c_ssq[:])
            cT = singles.tile([P, DCH, K], F32)
            for j in range(DCH):
                ps_ct = ps_xt_pool.tile([P, K], F32, tag="ps_xt")
                nc.tensor.transpose(
                    ps_ct[:], c_nat[:, j * P:(j + 1) * P], ident[:K, :K]
                )
                nc.scalar.copy(out=cT[:, j, :], in_=ps_ct[:])

            acc = singles.tile([P, g], F32)
            nc.vector.memset(acc[:], 0.0)

            # ---------------- main loop ----------------
            # Flat software pipeline over (macro, pair) units.  For each unit
            # we emit stats + transposes + PSUM->SBUF copies, then the raw
            # matmuls of the *previous* unit, so the PE stream alternates
            # short transpose bursts with dense matmul bursts (HAM-friendly)
            # with no matmul-sparse stretch at macro boundaries.  A macro's
            # tail (softmax etc.) is emitted right after its last matmuls.
            state: dict = {}

            def start_macro(m):
                r0 = m * g * P
                st = {}
                st["x_t"] = xin_pool.tile([P, g, D], F32, tag="x", name="x_t")
                src_ap = x[r0:r0 + g * P, :].rearrange("(p a) d -> p a d", p=P)
                step = pair if m == 0 else g // 2
                for a0 in range(0, g, step):
                    nc.sync.dma_start(
                        out=st["x_t"][:, a0:a0 + step, :],
                        in_=src_ap[:, a0:a0 + step, :],
                    )
                h = g // 2
                st["ps_raw"] = [
                    ps_raw_pool.tile([P, h * K], F32, tag="raw", name="ps_raw")
                    for _ in range(2)
                ]
                for t_ in ("ssq", "arg", "z", "t1", "rna", "s", "r", "lr"):
                    st[t_] = stage_pool.tile([P, g], F32, tag=t_, name=t_)
                st["gar"] = stage_pool.tile([P, D], F32, tag="gar", name="gar")
                st["e_t"] = ev_pool.tile([P, g, K], F32, tag="e", name="e")
                st["out_t"] = ev_pool.tile([P, g, K], F32, tag="po", name="po")
                st["pi2"] = ev_pool.tile([P, g, K], F32, tag="pi", name="pi")
                state[m] = st
                return st

            def emit_unit_front(m, q):
                st = state[m]
                x_t, ps_raw = st["x_t"], st["ps_raw"]
                xts = []
                for gi in range(q * pair, (q + 1) * pair):
                    xg = x_t[:, gi, :]
                    # one-pass row ssq (DVE)
                    nc.vector.scalar_tensor_tensor(
                        out=st["gar"][:], in0=xg, scalar=1.0, in1=xg,
                        op0=ALU.mult, op1=ALU.mult,
                        accum_out=st["ssq"][:, gi:gi + 1],
                    )
                    # transpose burst (PE)
                    ps_xt = ps_xt_pool.tile([P, D], F32, tag="ps_xt")
                    for j in range(DCH):
                        nc.tensor.transpose(
                            ps_xt[:, j * P:(j + 1) * P],
                            xg[:, j * P:(j + 1) * P],
                            ident[:],
                        )
                    xt_sb = xt_pool.tile([P, D], F32, tag="xt")
                    nc.scalar.copy(out=xt_sb[:], in_=ps_xt[:])
                    xts.append((gi, xt_sb))

                h = g // 2

                def emit_mms():
                    for gi, xt_sb in xts:
                        raw = ps_raw[gi // h][:, (gi % h) * K:(gi % h + 1) * K]
                        for j in range(DCH):
                            nc.tensor.matmul(
                                raw,
                                lhsT=xt_sb[:, j * P:(j + 1) * P],
                                rhs=cT[:, j, :],
                                start=(j == 0), stop=(j == DCH - 1),
                            )
                return emit_mms

            def emit_rn_chain(m, sl=slice(None)):
                # rsqrt: arg = c1*intbits(ssq) + lnA2 ; z0 = exp(arg);
                # Newton z <- z*(1.5 + (-0.5/a^2)*ssq*z^2)  (z ~ alpha/||x||)
                st = state[m]
                nc.vector.tensor_scalar(
                    out=st["arg"][:, sl], in0=st["ssq"][:, sl].bitcast(I32),
                    scalar1=float(-0.5 * LN2 / (1 << 23)), scalar2=lnA2[:],
                    op0=ALU.mult, op1=ALU.add,
                )
                nc.scalar.activation(st["z"][:, sl], st["arg"][:, sl], AF.Exp)
                for it in range(NEWTON_ITERS):
                    dst = st["rna"] if it == NEWTON_ITERS - 1 else st["z"]
                    nc.vector.tensor_mul(
                        st["t1"][:, sl], st["z"][:, sl], st["z"][:, sl]
                    )
                    nc.vector.tensor_mul(
                        st["t1"][:, sl], st["t1"][:, sl], st["ssq"][:, sl]
                    )
                    nc.vector.tensor_scalar(
                        out=st["t1"][:, sl], in0=st["t1"][:, sl],
                        scalar1=a_m05i2[:],
                        scalar2=1.5, op0=ALU.mult, op1=ALU.add,
                    )
                    nc.vector.tensor_mul(
                        dst[:, sl], st["z"][:, sl], st["t1"][:, sl]
                    )

            def emit_tail_half(m, hi):
                st = state[m] if hi == 0 else state.pop(m)
                r0 = m * g * P
                h = g // 2
                sl = slice(hi * h, (hi + 1) * h)
                raw = st["ps_raw"][hi]
                e_t, out_t, pi2 = st["e_t"], st["out_t"], st["pi2"]
                for li in range(h):
                    gi = hi * h + li
                    # e = exp(raw * alpha/||x||)  (ACT, PSUM src)
                    nc.scalar.activation(
                        out=e_t[:, gi, :],
                        in_=raw[:, li * K:(li + 1) * K],
                        func=AF.Exp,
                        scale=st["rna"][:, gi:gi + 1],
                    )
                # softmax denominators + reciprocal (DVE)
                nc.vector.tensor_reduce(
                    st["s"][:, sl], e_t[:, sl, :],
                    axis=mybir.AxisListType.X, op=ALU.add,
                )
                nc.vector.reciprocal(st["r"][:, sl], st["s"][:, sl])
                # probs = e * bcast(1/s)   (GPSIMD)
                r_bcast = st["r"][:, sl].unsqueeze(2).to_broadcast([P, h, K])
                nc.gpsimd.tensor_tensor(
                    out_t[:, sl, :], e_t[:, sl, :], r_bcast, op=ALU.mult
                )
                # loss: acc += rna * sum_k probs*raw   (DVE)
                nc.vector.tensor_mul(
                    pi2[:, sl, :].rearrange("p a k -> p (a k)"),
                    out_t[:, sl, :].rearrange("p a k -> p (a k)"),
                    raw[:],
                )
                nc.vector.tensor_reduce(
                    st["lr"][:, sl], pi2[:, sl, :],
                    axis=mybir.AxisListType.X, op=ALU.add,
                )
                nc.vector.tensor_mul(
                    st["lr"][:, sl], st["lr"][:, sl], st["rna"][:, sl]
                )
                nc.vector.tensor_add(acc[:, sl], acc[:, sl], st["lr"][:, sl])
                # probs out (ACT HWDGE ring): 2KB contiguous per partition
                pr = probs[r0:r0 + g * P, :].rearrange(
                    "(p b a) k -> p b a k", p=P, b=2
                )
                nc.scalar.dma_start(out=pr[:, hi], in_=out_t[:, sl, :])

            n_pairs = g // pair
            h = g // 2
            hp = n_pairs // 2  # pairs per half
            units = [(m, q) for m in range(n_mac) for q in range(n_pairs)]
            from collections import deque
            pend = deque()  # (m, q, emit_mms), depth-2 deferral
            half1_of = None  # macro whose tail-half-1 is deferred
            last = n_mac - 1

            def drain_one():
                nonlocal half1_of
                pm, pq, pfn = pend.popleft()
                pfn()
                if half1_of is not None:
                    emit_tail_half(half1_of, 1)
                    half1_of = None
                if pq == n_pairs - 1:
                    if pm != last:
                        emit_tail_half(pm, 0)
                    half1_of = pm
                elif pm == last and pq == hp - 1:
                    # last macro: first half fully matmul'd, rna(half0)
                    # ready -> drain it early
                    emit_tail_half(pm, 0)

            for m, q in units:
                if q == 0 and m not in state:
                    start_macro(m)
                emit_mms = emit_unit_front(m, q)
                if m == last and q == hp - 1:
                    emit_rn_chain(m, slice(0, h))
                elif m == last and q == n_pairs - 1:
                    emit_rn_chain(m, slice(h, g))
                elif m != last and q == n_pairs - 1:
                    emit_rn_chain(m)
                pend.append((m, q, emit_mms))
                if len(pend) > 2:
                    drain_one()
            while pend:
                drain_one()
            if half1_of is not None:
                emit_tail_half(half1_of, 1)

            # ---------------- loss partial ----------------
            accv = singles.tile([P, 1], F32)
            nc.vector.tensor_reduce(
                accv[:], acc[:], axis=mybir.AxisListType.X, op=ALU.add
            )
            ps_fin = ps_raw_pool.tile([1, 1], F32, tag="raw", name="ps_fin")
            nc.tensor.matmul(
                ps_fin[:], lhsT=accv[:], rhs=ones[:], start=True, stop=True
            )
            ls_sb = singles.tile([1, 1], F32)
            nc.scalar.copy(out=ls_sb[:], in_=ps_fin[:])
            nc.sync.dma_start(out=lsum[:, :], in_=ls_sb[:])

    nc.compile()
    return nc


_NC_CACHE: dict = {}


def _get_nc(rows: int):
    if rows not in _NC_CACHE:
        _NC_CACHE[rows] = build_nc(rows)
    return _NC_CACHE[rows]


def kernel(x, clusters, alpha):
    x = np.ascontiguousarray(np.asarray(x, dtype=np.float32))
    clusters = np.ascontiguousarray(np.asarray(clusters, dtype=np.float32))
    alpha_f = np.float32(np.asarray(alpha).reshape(()))

    n = x.shape[0]
    rows = n // N_CORES
    nc = _get_nc(rows)

    alphav = np.full((P, 1), alpha_f, dtype=np.float32)
    in_maps = [
        {"x": x[i * rows:(i + 1) * rows], "clusters": clusters, "alphav": alphav}
        for i in range(N_CORES)
    ]
    res = run_bass_kernel_spmd(nc, in_maps, list(range(N_CORES)))
    probs = np.concatenate([res.results[i]["probs"] for i in range(N_CORES)], axis=0)
    total = sum(float(res.results[i]["lsum"][0, 0]) for i in range(N_CORES))
    loss = np.float32(-total / (n * float(alpha_f)))
    return loss, probs
